# revision 13
# baseline (speedup 1.0000x reference)
"""FlashInfer-style GQA decode attention (B=32, L=8192, 16 q heads / 4 kv heads,
d=128) on 8 Trainium2 NeuronCores.

Sharding: 8-way data parallel over batch. Each core handles 4 sequences:
QKV projection, per-head RMSNorm, attention over its local KV cache (+ the
new token), and the output projection. Host concatenates per-core outputs.

Compute dtype: bf16 for TensorEngine operands (cast during DMA), fp32
accumulation in PSUM, softmax exp in fp32. Memory-bound: ~168 MB HBM reads
per core.
"""

import math
from dataclasses import dataclass

import numpy as np

import concourse.bass as bass
import concourse.tile as tile
from concourse import bacc, mybir
from concourse.bass_utils import run_bass_kernel_spmd
from concourse.masks import make_identity

F32 = mybir.dt.float32
BF16 = mybir.dt.bfloat16
Exp = mybir.ActivationFunctionType.Exp
Sqrt = mybir.ActivationFunctionType.Sqrt


@dataclass(frozen=True)
class Cfg:
    b_loc: int = 4          # sequences per core
    L: int = 8192           # cache length
    kvh: int = 4            # kv heads
    g: int = 4              # q heads per kv head
    d: int = 128            # head dim
    hid: int = 2048
    n_chunks: int = 4       # L split into chunks for DMA/SBUF tiling
    eps: float = 1e-6

    @property
    def nq(self):
        return self.kvh * self.g

    @property
    def qkv_cols(self):
        return (self.nq + 2 * self.kvh) * self.d

    @property
    def qk_cols(self):
        return (self.nq + self.kvh) * self.d

    @property
    def chunk_l(self):
        return self.L // self.n_chunks

    @property
    def tpc(self):
        return self.chunk_l // 128  # 128-row L-tiles per chunk

    @property
    def hk(self):
        return self.hid // 128  # 128-row k-tiles of the hidden dim


def build(cfg: Cfg) -> bacc.Bacc:
    nc = bacc.Bacc("TRN2", target_bir_lowering=False, debug=True)
    c = cfg
    scale = c.d ** -0.5

    hidden = nc.dram_tensor("hidden", [c.b_loc, c.hid], F32, kind="ExternalInput")
    k_cache = nc.dram_tensor("k_cache", [c.b_loc, c.L, c.kvh, c.d], F32,
                             kind="ExternalInput")
    v_cache = nc.dram_tensor("v_cache", [c.b_loc, c.L, c.kvh, c.d], F32,
                             kind="ExternalInput")
    w_qkv = nc.dram_tensor("w_qkv", [c.hid, c.qkv_cols], F32, kind="ExternalInput")
    w_o = nc.dram_tensor("w_o", [c.hid, c.hid], F32, kind="ExternalInput")
    b_row = nc.dram_tensor("b_row", [c.b_loc, c.qkv_cols], F32, kind="ExternalInput")
    gamma_row = nc.dram_tensor("gamma_row", [c.b_loc, c.qk_cols], F32,
                               kind="ExternalInput")
    out_d = nc.dram_tensor("out", [c.b_loc, c.hid], F32, kind="ExternalOutput")

    with tile.TileContext(nc) as tc:
        with (
            tc.tile_pool(name="consts", bufs=1) as consts,
            tc.tile_pool(name="wo_pool", bufs=1) as wo_pool,
        ):
            ident = consts.tile([128, 128], BF16)
            make_identity(nc, ident)
            ones_c = consts.tile([128, 1], BF16)
            nc.vector.memset(ones_c, 1.0)
            eps_sb = consts.tile([c.b_loc, 1], F32)
            nc.vector.memset(eps_sb, c.eps)

            # persistent small tensors
            qT_all = consts.tile([128, c.b_loc * c.nq], BF16)      # [d, s*16+j*4+g]
            kT_new = consts.tile([128, c.b_loc * c.kvh], BF16)     # [d, s*4+j]
            # new-token v / exp(scores) flattened on partition 0 (matmul
            # operands must start at partition 0)
            v_new = consts.tile([1, c.b_loc * c.kvh * (c.d + 1)], BF16)  # [v, 1]
            exp_new = consts.tile([1, c.b_loc * c.kvh * c.g], BF16)  # r-th g-block
            attnT = consts.tile([128, c.nq * c.b_loc], BF16)       # [d, h*4+s]

            # ---------------- Phase A: QKV projection + norms ----------------
            with (
                tc.tile_pool(name="phA", bufs=1) as phA,
                tc.tile_pool(name="wq_pool", bufs=3) as wq_pool,
                tc.tile_pool(name="psA", bufs=1, space="PSUM") as psA,
                tc.tile_pool(name="psA_small", bufs=1, space="PSUM") as psAs,
            ):
                # hidden^T in bf16: [128, hk, b_loc]
                hT = phA.tile([128, c.hk, c.b_loc], BF16)
                for k in range(c.hk):
                    nc.gpsimd.dma_start(
                        out=hT[:, k, :],
                        in_=hidden[:, k * 128:(k + 1) * 128].rearrange(
                            "s p -> p s"))

                b_sb = phA.tile([c.b_loc, c.qkv_cols], F32)
                nc.gpsimd.dma_start(out=b_sb, in_=b_row[:])
                gamma_sb = phA.tile([c.b_loc, c.qk_cols], F32)
                nc.gpsimd.dma_start(out=gamma_sb, in_=gamma_row[:])

                qkv_ps = psA.tile([c.b_loc, c.qkv_cols], F32)
                n_n = c.qkv_cols // 512
                for k in range(c.hk):
                    wq_t = wq_pool.tile([128, c.qkv_cols], BF16, tag="wq")
                    nc.gpsimd.dma_start(
                        out=wq_t, in_=w_qkv[k * 128:(k + 1) * 128, :])
                    for n in range(n_n):
                        nc.tensor.matmul(
                            qkv_ps[:, n * 512:(n + 1) * 512],
                            hT[:, k, :],
                            wq_t[:, n * 512:(n + 1) * 512],
                            start=(k == 0), stop=(k == c.hk - 1))

                qkv_sb = phA.tile([c.b_loc, c.qkv_cols], F32)
                nc.vector.tensor_add(out=qkv_sb, in0=qkv_ps, in1=b_sb)

                # RMSNorm over q+k segments
                nh = c.nq + c.kvh
                sq = phA.tile([c.b_loc, c.qk_cols], F32)
                nc.vector.tensor_mul(out=sq, in0=qkv_sb[:, :c.qk_cols],
                                     in1=qkv_sb[:, :c.qk_cols])
                var = phA.tile([c.b_loc, nh], F32)
                nc.vector.reduce_sum(
                    out=var, in_=sq.rearrange("s (h d) -> s h d", d=c.d),
                    axis=mybir.AxisListType.X)
                nc.scalar.activation(out=var, in_=var, func=Sqrt,
                                     bias=eps_sb, scale=1.0 / c.d)
                nc.vector.reciprocal(out=var, in_=var)  # rstd [b_loc, nh]

                qkn = phA.tile([c.b_loc, c.qk_cols], BF16)
                tmp = phA.tile([c.b_loc, c.qk_cols], F32)
                nc.vector.tensor_mul(out=tmp, in0=qkv_sb[:, :c.qk_cols],
                                     in1=gamma_sb)
                rstd_b = bass.AP(tensor=var.tensor, offset=var.offset,
                                 ap=[var.ap[0], var.ap[1], [0, c.d]])
                nc.vector.tensor_mul(
                    out=qkn.rearrange("s (h d) -> s h d", d=c.d),
                    in0=tmp.rearrange("s (h d) -> s h d", d=c.d),
                    in1=rstd_b)

                # v of the new token (not normed): bf16
                v_nat = phA.tile([c.b_loc, c.kvh * c.d], BF16)
                nc.scalar.copy(out=v_nat, in_=qkv_sb[:, c.qk_cols:])

                # transposes of q/k head chunks -> qT_all / kT_new columns
                qT3 = qT_all.rearrange("p (s h) -> p s h", h=c.nq)
                kT3 = kT_new.rearrange("p (s j) -> p s j", j=c.kvh)
                for ch in range(nh):
                    trp = psAs.tile([128, c.b_loc], BF16, tag="trA")
                    nc.tensor.transpose(
                        trp, qkn[:, ch * c.d:(ch + 1) * c.d],
                        ident[:c.b_loc, :c.b_loc])
                    if ch < c.nq:
                        nc.scalar.copy(out=qT3[:, :, ch], in_=trp)
                    else:
                        nc.scalar.copy(out=kT3[:, :, ch - c.nq], in_=trp)

                # new-token v blocks gathered onto partition 0 (SBUF->SBUF DMA
                # moves across partitions)
                vnb = c.d + 1
                for s in range(c.b_loc):
                    for j in range(c.kvh):
                        r = s * c.kvh + j
                        nc.gpsimd.dma_start(
                            out=v_new[0:1, r * vnb:r * vnb + c.d],
                            in_=v_nat[s:s + 1, j * c.d:(j + 1) * c.d])
                nc.vector.memset(
                    v_new.rearrange("o (r e) -> o r e", e=vnb)[:, :, c.d], 1.0)

                # new-token scores, one [1, g] matmul per (s, j), all on
                # partition 0
                stn = psAs.tile([1, c.b_loc * c.kvh * c.g], F32, tag="stn")
                for s in range(c.b_loc):
                    for j in range(c.kvh):
                        r = s * c.kvh + j
                        qcol = s * c.nq + j * c.g
                        nc.tensor.matmul(
                            stn[:, r * c.g:(r + 1) * c.g],
                            kT_new[:, r:r + 1], qT_all[:, qcol:qcol + c.g],
                            start=True, stop=True)
                nc.scalar.activation(out=exp_new, in_=stn, func=Exp,
                                     scale=scale)

            # w_o resident, prefetched during attention
            wo_all = wo_pool.tile([128, c.hk, c.hid], BF16)
            nc.gpsimd.dma_start(
                out=wo_all, in_=w_o.rearrange("(k p) n -> p k n", p=128))

            # ---------------- Phase B: attention over the cache ----------------
            with (
                tc.tile_pool(name="kv_pool", bufs=2) as kv_pool,
                tc.tile_pool(name="kt_pool", bufs=4) as kt_pool,
                tc.tile_pool(name="exp_pool", bufs=2) as exp_pool,
                tc.tile_pool(name="small", bufs=4) as small,
                tc.tile_pool(name="ps_st", bufs=1, space="PSUM") as ps_st,
                tc.tile_pool(name="ps_kt", bufs=2, space="PSUM") as ps_kt,
                tc.tile_pool(name="ps_o", bufs=c.kvh, space="PSUM") as ps_o,
                tc.tile_pool(name="ps_tr", bufs=1, space="PSUM") as ps_tr,
            ):
                att3 = attnT.rearrange("p (h s) -> p h s", s=c.b_loc)
                vnb = c.d + 1
                for s in range(c.b_loc):
                    # one PSUM bank per kv head: cols 0..127 = p@V, col 128 =
                    # sum(p); single accumulation group per bank over all
                    # chunks + the new token
                    o_part = [ps_o.tile([c.g, c.d + 1], F32, tag="o",
                                        name=f"o_part_{s}_{j}")
                              for j in range(c.kvh)]
                    for lc in range(c.n_chunks):
                        lo = lc * c.chunk_l
                        # (p t) split: each partition reads a contiguous
                        # 32KB span of the cache (softmax is invariant to
                        # the position permutation)
                        kc = kv_pool.tile([128, c.tpc, c.kvh, c.d], BF16,
                                          tag="k")
                        nc.gpsimd.dma_start(
                            out=kc,
                            in_=k_cache[s, lo:lo + c.chunk_l, :, :].rearrange(
                                "(p t) j d -> p t j d", p=128))
                        vc = kv_pool.tile([128, c.tpc, c.kvh, c.d], BF16,
                                          tag="v")
                        nc.gpsimd.dma_start(
                            out=vc,
                            in_=v_cache[s, lo:lo + c.chunk_l, :, :].rearrange(
                                "(p t) j d -> p t j d", p=128))

                        st_ps = ps_st.tile([128, c.tpc, c.kvh, c.g], F32,
                                           tag="st")
                        for t in range(c.tpc):
                            for j in range(c.kvh):
                                ktp = ps_kt.tile([128, c.d], BF16, tag="ktp")
                                nc.tensor.transpose(ktp, kc[:, t, j, :], ident)
                                kts = kt_pool.tile([128, c.d], BF16, tag="kts")
                                if (t * c.kvh + j) % 2 == 0:
                                    nc.scalar.copy(out=kts, in_=ktp)
                                else:
                                    nc.vector.tensor_copy(out=kts, in_=ktp)
                                qcol = s * c.nq + j * c.g
                                nc.tensor.matmul(
                                    st_ps[:, t, j, :], kts,
                                    qT_all[:, qcol:qcol + c.g],
                                    start=True, stop=True)
                        et = exp_pool.tile([128, c.tpc, c.kvh, c.g], BF16,
                                           tag="et")
                        nc.scalar.activation(out=et, in_=st_ps, func=Exp,
                                             scale=scale)
                        first = lc == 0
                        for j in range(c.kvh):
                            for t in range(c.tpc):
                                nc.tensor.matmul(
                                    o_part[j][:, :c.d], et[:, t, j, :],
                                    vc[:, t, j, :],
                                    start=(first and t == 0), stop=False)
                                nc.tensor.matmul(
                                    o_part[j][:, c.d:c.d + 1], et[:, t, j, :],
                                    ones_c, start=False, stop=False,
                                    skip_group_check=True)

                    # new token, normalize, stage attn^T
                    for j in range(c.kvh):
                        r = s * c.kvh + j
                        en = exp_new[0:1, r * c.g:(r + 1) * c.g]
                        nc.tensor.matmul(o_part[j], en,
                                         v_new[0:1, r * vnb:(r + 1) * vnb],
                                         start=False, stop=True)
                        recip = small.tile([c.g, 1], F32, tag="recip")
                        nc.vector.reciprocal(out=recip,
                                             in_=o_part[j][:, c.d:c.d + 1])
                        o_n = small.tile([c.g, c.d], BF16, tag="o_n")
                        nc.vector.tensor_scalar_mul(
                            out=o_n, in0=o_part[j][:, :c.d], scalar1=recip)
                        trp = ps_tr.tile([c.d, c.g], BF16, tag="trB")
                        nc.tensor.transpose(trp, o_n, ident[:c.g, :c.g])
                        nc.scalar.copy(out=att3[:, j * c.g:(j + 1) * c.g, s],
                                       in_=trp)

            # ---------------- Phase C: output projection ----------------
            with (
                tc.tile_pool(name="phC", bufs=1) as phC,
                tc.tile_pool(name="psC", bufs=1, space="PSUM") as psC,
            ):
                out_ps = psC.tile([c.b_loc, c.hid], F32)
                for k in range(c.hk):
                    for n in range(c.hid // 512):
                        nc.tensor.matmul(
                            out_ps[:, n * 512:(n + 1) * 512],
                            att3[:, k, :],
                            wo_all[:, k, n * 512:(n + 1) * 512],
                            start=(k == 0), stop=(k == c.hk - 1))
                out_sb = phC.tile([c.b_loc, c.hid], F32)
                nc.vector.tensor_copy(out=out_sb, in_=out_ps)
                nc.gpsimd.dma_start(out=out_d[:], in_=out_sb)

    nc.compile()
    return nc


def make_host_inputs(cfg: Cfg, hidden, k_cache, v_cache, w_qkv, b_qkv, w_o,
                     q_gamma, k_gamma, n_cores):
    """Shard full inputs into per-core in_maps (data-parallel over batch)."""
    c = cfg
    hidden = np.asarray(hidden, np.float32).reshape(-1, c.hid)
    b_row = np.ascontiguousarray(
        np.broadcast_to(np.asarray(b_qkv, np.float32), (c.b_loc, c.qkv_cols)))
    gamma = np.concatenate([
        np.tile(np.asarray(q_gamma, np.float32), c.nq),
        np.tile(np.asarray(k_gamma, np.float32), c.kvh)])
    gamma_row = np.ascontiguousarray(
        np.broadcast_to(gamma, (c.b_loc, c.qk_cols)))
    w_qkv = np.asarray(w_qkv, np.float32)
    w_o = np.asarray(w_o, np.float32)
    k_cache = np.asarray(k_cache, np.float32)
    v_cache = np.asarray(v_cache, np.float32)
    in_maps = []
    for core in range(n_cores):
        sl = slice(core * c.b_loc, (core + 1) * c.b_loc)
        in_maps.append({
            "hidden": hidden[sl],
            "k_cache": k_cache[sl],
            "v_cache": v_cache[sl],
            "w_qkv": w_qkv,
            "w_o": w_o,
            "b_row": b_row,
            "gamma_row": gamma_row,
        })
    return in_maps


_CACHE = {}


def _get_nc(cfg: Cfg):
    if cfg not in _CACHE:
        _CACHE[cfg] = build(cfg)
    return _CACHE[cfg]


def kernel(hidden_states, k_cache, v_cache, w_qkv, b_qkv, w_o, q_gamma,
           k_gamma):
    B = hidden_states.shape[0]
    n_cores = 8
    cfg = Cfg(b_loc=B // n_cores, L=k_cache.shape[1])
    nc = _get_nc(cfg)
    in_maps = make_host_inputs(cfg, hidden_states, k_cache, v_cache, w_qkv,
                               b_qkv, w_o, q_gamma, k_gamma, n_cores)
    res = run_bass_kernel_spmd(nc, in_maps, core_ids=list(range(n_cores)))
    out = np.concatenate([r["out"] for r in res.results], axis=0)
    return out.reshape(B, 1, hidden_states.shape[-1]).astype(np.float32)
